# revision 4
# baseline (speedup 1.0000x reference)
"""Llama-3-8B-style GQA attention layer (bsz=1, seq=2048, dim=4096) on 8 TRN2 NeuronCores.

Tensor-parallel over heads: core i owns Q heads 4i..4i+3 and KV head i.

Fused pipeline (single static pool set, no per-rep pool barriers):
  Block sb's QKV projections (3 psum-light passes: {q0,q1},{q2,q3},{k,v}) are
  interleaved in program order with attention of block sb-1, so the PE fills
  softmax-latency stalls with projection matmuls and stays HAM-warm.
  Attention q-block 3 is interleaved with the (flipped) wo GEMM half 0.
  Stage C: AllGather of normalized O^T (bf16) in two s-halves; AG1 overlaps
  blocks 2-3, AG2 overlaps wo-half-0.
  Stage D: wo GEMM flipped (wo chunk stationary, O^T moving) producing
  out^T[oc, s] with 4 psum accumulators; host transposes.
  Softmax denominator: rank-1 PE matmuls -> reciprocal_approx_fast on the
  [1,512] row -> bf16 -> gpsimd partition_broadcast -> one DVE multiply.
"""
import numpy as np
import ml_dtypes

BF16 = ml_dtypes.bfloat16
N_CORES = 8
SEQ = 2048
DIM = 4096
HD = 128          # head dim
NQH = 4           # Q heads per core
QCOLS = NQH * HD  # 512
SM_SCALE = 1.0 / float(np.sqrt(HD))

_cache = {}


def _build_nc(reps: int = 1):
    import concourse.bacc as bacc
    import concourse.mybir as mybir
    import concourse.tile as tile
    import concourse.masks as masks

    dt = mybir.dt
    Alu = mybir.AluOpType
    Act = mybir.ActivationFunctionType

    nc = bacc.Bacc("TRN2", target_bir_lowering=False, debug=False)

    xT_e = nc.declare_dram_parameter("xT", [DIM, SEQ], dt.bfloat16, isOutput=False)
    wq_e = nc.declare_dram_parameter("wq", [DIM, QCOLS], dt.bfloat16, isOutput=False)
    wk_e = nc.declare_dram_parameter("wk", [DIM, HD], dt.bfloat16, isOutput=False)
    wv_e = nc.declare_dram_parameter("wv", [DIM, HD], dt.bfloat16, isOutput=False)
    wo_e = nc.declare_dram_parameter("wo", [DIM, QCOLS], dt.bfloat16, isOutput=False)
    cs_e = nc.declare_dram_parameter("cs", [256, SEQ], dt.bfloat16, isOutput=False)
    # out is transposed: out^T[oc, s]; host transposes back
    out_e = nc.declare_dram_parameter("out", [QCOLS, SEQ], dt.float32, isOutput=True)

    ag1 = nc.dram_tensor("ag1", [DIM, 1024], dt.bfloat16, addr_space="Shared")
    ag2 = nc.dram_tensor("ag2", [DIM, 1024], dt.bfloat16, addr_space="Shared")

    NSB = SEQ // 512   # 4 seq blocks of 512
    NCH = DIM // 128   # 32 contraction chunks
    NG = NCH // 4      # 8 four-chunk groups

    with tile.TileContext(nc) as tc:
        with (
            tc.tile_pool(name="persist", bufs=1) as pp,
            tc.tile_pool(name="dram", bufs=1, space="DRAM") as dramp,
            tc.tile_pool(name="xtp", bufs=8) as xtp,
            tc.tile_pool(name="qbfp", bufs=2) as qbfp,
            tc.tile_pool(name="vtmp", bufs=2) as vtp,
            tc.tile_pool(name="ptp", bufs=5) as ptp,
            tc.tile_pool(name="denp", bufs=2) as denp,
            tc.tile_pool(name="recp", bufs=2) as recp,
            tc.tile_pool(name="atp", bufs=3) as atp,
            tc.tile_pool(name="outp", bufs=2) as outp,
            tc.tile_pool(name="psumA", bufs=2, space="PSUM") as psA,
            tc.tile_pool(name="psumS", bufs=2, space="PSUM") as psS,
            tc.tile_pool(name="psumO", bufs=2, space="PSUM") as psO,
            tc.tile_pool(name="psumW", bufs=2, space="PSUM") as psW,
        ):
            # ---- persistent SBUF tensors ----
            wq_g = [pp.tile([128, 4 * QCOLS], dt.bfloat16, name=f"wqg{g}") for g in range(NG)]
            wk_g = [pp.tile([128, 4 * HD], dt.bfloat16, name=f"wkg{g}") for g in range(NG)]
            wv_g = [pp.tile([128, 4 * HD], dt.bfloat16, name=f"wvg{g}") for g in range(NG)]
            wo_g = [pp.tile([128, 4 * QCOLS], dt.bfloat16, name=f"wog{g}") for g in range(NG)]
            wq_sb = [wq_g[c // 4][:, (c % 4) * QCOLS:(c % 4 + 1) * QCOLS] for c in range(NCH)]
            wk_sb = [wk_g[c // 4][:, (c % 4) * HD:(c % 4 + 1) * HD] for c in range(NCH)]
            wv_sb = [wv_g[c // 4][:, (c % 4) * HD:(c % 4 + 1) * HD] for c in range(NCH)]
            wo_sb = [wo_g[c // 4][:, (c % 4) * QCOLS:(c % 4 + 1) * QCOLS] for c in range(NCH)]
            cos_sb = pp.tile([128, SEQ], dt.bfloat16)     # cos duplicated in both halves
            sin_sb = pp.tile([128, SEQ], dt.bfloat16)     # sin duplicated in both halves
            tri01 = pp.tile([128, 128], dt.bfloat16)      # 1 iff k <= q
            ident = pp.tile([128, 128], dt.bfloat16)
            ones_col = pp.tile([128, 1], dt.bfloat16)
            # per-block RoPE'd tensors (exact deps for the fused schedule)
            qrope_t = [[pp.tile([128, 512], dt.bfloat16, name=f"qr{h}_{sb}")
                        for sb in range(NSB)] for h in range(NQH)]
            krope_t = [pp.tile([128, 512], dt.bfloat16, name=f"kr{sb}") for sb in range(NSB)]
            v_t = [pp.tile([128, 512], dt.bfloat16, name=f"vt{sb}") for sb in range(NSB)]
            oTh = [[pp.tile([128, 1024], dt.bfloat16, name=f"oT{h}_{half}")
                    for half in range(2)] for h in range(NQH)]

            for g in range(NG):
                gsl = slice(g * 512, (g + 1) * 512)
                nc.scalar.dma_start(wq_g[g][:].rearrange("p (c m) -> p c m", c=4),
                                    wq_e.ap()[gsl, :].rearrange("(c p) m -> p c m", p=128))
                nc.scalar.dma_start(wk_g[g][:].rearrange("p (c m) -> p c m", c=4),
                                    wk_e.ap()[gsl, :].rearrange("(c p) m -> p c m", p=128))
                nc.scalar.dma_start(wv_g[g][:].rearrange("p (c m) -> p c m", c=4),
                                    wv_e.ap()[gsl, :].rearrange("(c p) m -> p c m", p=128))
                if g == 0:
                    nc.scalar.dma_start(cos_sb[:], cs_e.ap()[0:128, :])
                    nc.scalar.dma_start(sin_sb[:], cs_e.ap()[128:256, :])

            nc.gpsimd.memset(tri01[:], 1.0)
            nc.gpsimd.affine_select(
                out=tri01[:], in_=tri01[:], compare_op=Alu.is_ge, fill=0.0,
                base=0, pattern=[[1, 128]], channel_multiplier=-1,
            )
            masks.make_identity(nc, ident[:])
            nc.gpsimd.memset(ones_col[:], 1.0)

            agins = [dramp.tile([QCOLS, 1024], dt.bfloat16, name=f"agin{i}")
                     for i in range(2)]

            for _rep in range(reps):
                # ======== per-rep emission via interleaved unit lists ========

                xts = {}  # (sb) -> list of 8 xt4 tiles

                def a_units(sb):
                    """Yield closures for block sb's QKV projections (3 passes)."""
                    sl = slice(sb * 512, (sb + 1) * 512)
                    units = []

                    def mk_pass(p):
                        # pass targets: 0 -> q0,q1 ; 1 -> q2,q3 ; 2 -> k,v
                        state = {}

                        def start():
                            state["t0"] = psA.tile([128, 512], dt.float32, name="qk")
                            state["t1"] = psA.tile([128, 512], dt.float32, name="qk")

                        def group(g):
                            def run():
                                if p == 0:
                                    if g == 0:
                                        start()
                                        xts[sb] = [None] * NG
                                    xt4 = xtp.tile([128, 4 * 512], dt.bfloat16, name="xt4")
                                    xts[sb][g] = xt4
                                    nc.sync.dma_start(
                                        xt4[:].rearrange("p (c s) -> p c s", c=4),
                                        xT_e.ap()[g * 512:(g + 1) * 512, sl]
                                        .rearrange("(c p) s -> p c s", p=128))
                                elif g == 0:
                                    start()
                                xt4 = xts[sb][g]
                                for cc in range(4):
                                    c = g * 4 + cc
                                    xt = xt4[:, cc * 512:(cc + 1) * 512]
                                    st, sp = (c == 0), (c == NCH - 1)
                                    if p == 0:
                                        w0s, w1s = wq_sb[c][:, 0:128], wq_sb[c][:, 128:256]
                                    elif p == 1:
                                        w0s, w1s = wq_sb[c][:, 256:384], wq_sb[c][:, 384:512]
                                    else:
                                        w0s, w1s = wk_sb[c], wv_sb[c]
                                    nc.tensor.matmul(state["t0"][:], w0s, xt, start=st,
                                                     stop=sp, skip_group_check=True)
                                    nc.tensor.matmul(state["t1"][:], w1s, xt, start=st,
                                                     stop=sp, skip_group_check=True)
                            return run

                        def rope(ps_key, dst):
                            def run():
                                ps = state[ps_key]
                                qbf = qbfp.tile([128, 512], dt.bfloat16, name="qbf")
                                nc.scalar.copy(qbf[:], ps[:])
                                tr_c = qbfp.tile([64, 512], dt.bfloat16, name="tr_c")
                                ti_s = qbfp.tile([64, 512], dt.bfloat16, name="ti_s")
                                tr_s = qbfp.tile([64, 512], dt.bfloat16, name="tr_s")
                                ti_c = qbfp.tile([64, 512], dt.bfloat16, name="ti_c")
                                nc.vector.tensor_mul(tr_c[:], qbf[0:64, :], cos_sb[0:64, sl])
                                nc.vector.tensor_mul(ti_s[:], qbf[64:128, :], sin_sb[64:128, sl])
                                nc.vector.tensor_sub(dst[0:64, :], tr_c[:], ti_s[:])
                                nc.vector.tensor_mul(tr_s[:], qbf[0:64, :], sin_sb[0:64, sl])
                                nc.vector.tensor_mul(ti_c[:], qbf[64:128, :], cos_sb[64:128, sl])
                                nc.vector.tensor_add(dst[64:128, :], tr_s[:], ti_c[:])
                            return run

                        def vtrans():
                            def run():
                                vT_sb = vtp.tile([128, 512], dt.bfloat16, name="vT_sb")
                                nc.scalar.copy(vT_sb[:], state["t1"][:])
                                for t in range(4):
                                    tp = psS.tile([128, 128], dt.bfloat16, name="sps")
                                    nc.tensor.transpose(tp[:], vT_sb[:, t * 128:(t + 1) * 128],
                                                        ident[:])
                                    nc.scalar.copy(v_t[sb][:, t * 128:(t + 1) * 128], tp[:])
                            return run

                        for g in range(NG):
                            units.append(group(g))
                        if p == 0:
                            units.append(rope("t0", qrope_t[0][sb]))
                            units.append(rope("t1", qrope_t[1][sb]))
                        elif p == 1:
                            units.append(rope("t0", qrope_t[2][sb]))
                            units.append(rope("t1", qrope_t[3][sb]))
                        else:
                            units.append(rope("t0", krope_t[sb]))
                            units.append(vtrans())

                    for p in range(3):
                        mk_pass(p)
                    return units

                def b_units(qb):
                    """Yield closures for attention of q-block qb (4 heads serial)."""
                    n_k = 4 * (qb + 1)
                    half = qb // 2
                    lql = slice((qb % 2) * 512, (qb % 2) * 512 + 512)
                    units = []

                    def mk_head(h):
                        state = {}

                        def step(kt):
                            def run():
                                if kt == 0:
                                    state["ops"] = psO.tile([128, 512], dt.float32, name="ops")
                                    state["dacc"] = [None, None]
                                ops = state["ops"]
                                o_idx = kt - 4 * qb
                                w0 = 128 * o_idx if o_idx > 0 else 0
                                wsl = slice(w0, 512)
                                sps = psS.tile([128, 512], dt.float32, name="sps")
                                nc.tensor.matmul(
                                    sps[:, wsl],
                                    krope_t[kt // 4][:, (kt % 4) * 128:(kt % 4 + 1) * 128],
                                    qrope_t[h][qb][:, wsl], start=True, stop=True,
                                    skip_group_check=True)
                                pt = ptp.tile([128, 512], dt.bfloat16, name="pt")
                                nc.scalar.activation(pt[:, wsl], sps[:, wsl], Act.Exp,
                                                     scale=SM_SCALE)
                                if o_idx >= 0:
                                    nc.vector.tensor_mul(pt[:, w0:w0 + 128],
                                                         pt[:, w0:w0 + 128], tri01[:])
                                nc.tensor.matmul(
                                    ops[:, wsl],
                                    v_t[kt // 4][:, (kt % 4) * 128:(kt % 4 + 1) * 128],
                                    pt[:, wsl], start=(kt == 0), stop=(kt == n_k - 1),
                                    skip_group_check=True)
                                j = kt % 2 if qb > 0 else 0
                                first = (kt == j) if qb > 0 else (kt == 0)
                                if first:
                                    d = denp.tile([128, 512], dt.bfloat16, name=f"dacc{j}")
                                    state["dacc"][j] = d
                                    nc.vector.tensor_copy(d[:], pt[:])
                                else:
                                    d = state["dacc"][j]
                                    nc.vector.tensor_add(d[:, wsl], d[:, wsl], pt[:, wsl])
                            return run

                        def finalize():
                            def run():
                                ops = state["ops"]
                                dacc = state["dacc"]
                                dsum = psS.tile([128, 512], dt.float32, name="sps")
                                nc.tensor.matmul(dsum[0:1, :], ones_col[:], dacc[0][:],
                                                 start=True, stop=(dacc[1] is None),
                                                 skip_group_check=True)
                                if dacc[1] is not None:
                                    nc.tensor.matmul(dsum[0:1, :], ones_col[:], dacc[1][:],
                                                     start=False, stop=True,
                                                     skip_group_check=True)
                                rrow = recp.tile([1, 512], dt.float32, name="rrow")
                                nc.vector.reciprocal_approx_fast(rrow[:], dsum[0:1, :])
                                rbf = recp.tile([1, 512], dt.bfloat16, name="rbf")
                                nc.vector.tensor_copy(rbf[:], rrow[:])
                                dbc = recp.tile([128, 512], dt.bfloat16, name="dbc")
                                nc.gpsimd.partition_broadcast(dbc[:], rbf[:])
                                nc.vector.tensor_mul(oTh[h][half][:, lql], ops[:], dbc[:])
                                if qb == 1 or qb == 3:
                                    nc.sync.dma_start(
                                        agins[half][h * 128:(h + 1) * 128, :],
                                        oTh[h][half][:])
                            return run

                        for kt in range(n_k):
                            units.append(step(kt))
                        units.append(finalize())

                    for h in range(NQH):
                        mk_head(h)
                    return units

                def ag_unit(half):
                    def run():
                        agdst = ag1 if half == 0 else ag2
                        nc.gpsimd.collective_compute(
                            "AllGather",
                            mybir.AluOpType.bypass,
                            replica_groups=[list(range(N_CORES))],
                            ins=[agins[half].opt()],
                            outs=[agdst[:]],
                        )
                    return run

                def d_units(half):
                    """Flipped wo GEMM: out^T[oc, s] = wo^T @ O^T, 4 accumulators."""
                    ag = ag1 if half == 0 else ag2
                    units = []

                    def mk_qseg(qs):
                        state = {}

                        def group(g):
                            def run():
                                if g == 0:
                                    state["w"] = [
                                        psA.tile([128, 512], dt.float32, name="qk"),
                                        psA.tile([128, 512], dt.float32, name="qk"),
                                        psW.tile([128, 512], dt.float32, name="wops"),
                                        psW.tile([128, 512], dt.float32, name="wops"),
                                    ]
                                at4 = atp.tile([128, 4 * 512], dt.bfloat16, name="at4")
                                nc.sync.dma_start(
                                    at4[:].rearrange("p (c s) -> p c s", c=4),
                                    ag[g * 512:(g + 1) * 512, qs * 512:(qs + 1) * 512]
                                    .rearrange("(c p) s -> p c s", p=128))
                                for cc in range(4):
                                    c = g * 4 + cc
                                    at = at4[:, cc * 512:(cc + 1) * 512]
                                    st, sp = (c == 0), (c == NCH - 1)
                                    for oc in range(4):
                                        nc.tensor.matmul(
                                            state["w"][oc][:],
                                            wo_sb[c][:, oc * 128:(oc + 1) * 128],
                                            at, start=st, stop=sp, skip_group_check=True)
                            return run

                        def drain(oc):
                            def run():
                                outsb = outp.tile([128, 512], dt.float32, name="outsb")
                                nc.scalar.copy(outsb[:], state["w"][oc][:])
                                col0 = half * 1024 + qs * 512
                                nc.scalar.dma_start(
                                    out_e.ap()[oc * 128:(oc + 1) * 128, col0:col0 + 512],
                                    outsb[:])
                            return run

                        for g in range(NG):
                            units.append(group(g))
                        for oc in range(4):
                            units.append(drain(oc))

                    for qs in range(2):
                        mk_qseg(qs)
                    return units

                def weave(primary, secondary):
                    """Merge two unit lists evenly, preserving each list's order."""
                    if not secondary:
                        return list(primary)
                    out = []
                    n, m = len(primary), len(secondary)
                    si = 0
                    for i, u in enumerate(primary):
                        out.append(u)
                        want = (i + 1) * m // n
                        while si < want:
                            out.append(secondary[si])
                            si += 1
                    out.extend(secondary[si:])
                    return out

                # phase 0: A0
                for u in a_units(0):
                    u()
                # phases 1..3: A(sb) woven with B(sb-1)
                for sb in range(1, NSB):
                    for u in weave(a_units(sb), b_units(sb - 1)):
                        u()
                    if sb == 2:
                        ag_unit(0)()
                    if _rep == 0 and sb == 3:
                        for g in range(NG):
                            nc.scalar.dma_start(
                                wo_g[g][:].rearrange("p (c m) -> p c m", c=4),
                                wo_e.ap()[g * 512:(g + 1) * 512, :]
                                .rearrange("(c p) m -> p c m", p=128))
                # phase 4: D half 0 woven with B3 (B3 front-loaded)
                dh0 = d_units(0)
                b3 = b_units(3)
                front, rest = dh0[:12], dh0[12:]
                for u in weave(front, b3):
                    u()
                ag_unit(1)()
                for u in rest:
                    u()
                # phase 5: D half 1
                for u in d_units(1):
                    u()

    nc.compile()
    return nc


def _prep_inputs(x, wq, wk, wv, wo):
    """Host-side sharding/layout prep. Returns per-core in_maps."""
    x2 = np.asarray(x, dtype=np.float32).reshape(SEQ, DIM)
    xT = np.ascontiguousarray(x2.T).astype(BF16)

    # permutation: within each head, even dims then odd dims (RoPE pair layout)
    perm_head = np.concatenate([np.arange(0, HD, 2), np.arange(1, HD, 2)])
    qperm = np.concatenate([g * HD + perm_head for g in range(32)])   # 32 Q heads
    kperm = np.concatenate([g * HD + perm_head for g in range(8)])    # 8 KV heads
    wq_p = np.asarray(wq, dtype=np.float32)[:, qperm].astype(BF16)
    wk_p = np.asarray(wk, dtype=np.float32)[:, kperm].astype(BF16)
    wv_b = np.asarray(wv, dtype=np.float32).astype(BF16)
    wo_b = np.asarray(wo, dtype=np.float32).astype(BF16)

    # RoPE tables: cos/sin[j, s], j = pair index 0..63
    inv_freq = 1.0 / (10000.0 ** (np.arange(0, HD, 2, dtype=np.float64) / HD))
    ang = inv_freq[:, None] * np.arange(SEQ, dtype=np.float64)[None, :]
    cosd = np.cos(ang)
    sind = np.sin(ang)
    cs = np.concatenate([cosd, cosd, sind, sind]).astype(BF16)

    in_maps = []
    for i in range(N_CORES):
        in_maps.append({
            "xT": xT,
            "wq": np.ascontiguousarray(wq_p[:, i * QCOLS:(i + 1) * QCOLS]),
            "wk": np.ascontiguousarray(wk_p[:, i * HD:(i + 1) * HD]),
            "wv": np.ascontiguousarray(wv_b[:, i * HD:(i + 1) * HD]),
            "wo": np.ascontiguousarray(wo_b[:, i * QCOLS:(i + 1) * QCOLS]),
            "cs": cs,
        })
    return in_maps


def _get_nc(reps: int = 1):
    key = ("nc", reps)
    if key not in _cache:
        _cache[key] = _build_nc(reps)
    return _cache[key]


def unshard(per_core_out):
    """per_core_out: list of 8 arrays [QCOLS, SEQ] (out^T) -> [1, SEQ, DIM]."""
    outT = np.concatenate(per_core_out, axis=0)  # [DIM, SEQ]
    return np.ascontiguousarray(outT.T).reshape(1, SEQ, DIM).astype(np.float32)


def kernel(x, wq, wk, wv, wo, start_pos=0, **_ignored):
    from concourse.bass_utils import run_bass_kernel_spmd

    nc = _get_nc()
    in_maps = _prep_inputs(x, wq, wk, wv, wo)
    res = run_bass_kernel_spmd(nc, in_maps, core_ids=list(range(N_CORES)))
    return unshard([res.results[i]["out"] for i in range(N_CORES)])


# revision 10
# speedup vs baseline: 1.1766x; 1.1766x over previous
"""Llama-3-8B-style GQA attention layer (bsz=1, seq=2048, dim=4096) on 8 TRN2 NeuronCores.

Tensor-parallel over heads: core i owns Q heads 4i..4i+3 and KV head i.

Fused pipeline (single static pool set, no per-rep pool barriers):
  Block sb's QKV projections (3 psum-light passes: {q0,q1},{q2,q3},{k,v}) are
  interleaved in program order with attention of block sb-1, so the PE fills
  softmax-latency stalls with projection matmuls and stays HAM-warm.
  Attention q-block 3 is interleaved with the (flipped) wo GEMM half 0.
  Stage C: AllGather of normalized O^T (bf16) in two s-halves; AG1 overlaps
  blocks 2-3, AG2 overlaps wo-half-0.
  Stage D: wo GEMM flipped (wo chunk stationary, O^T moving) producing
  out^T[oc, s] with 4 psum accumulators; host transposes.
  Softmax denominator: rank-1 PE matmuls -> reciprocal_approx_fast on the
  [1,512] row -> bf16 -> gpsimd partition_broadcast -> one DVE multiply.
"""
import numpy as np
import ml_dtypes

BF16 = ml_dtypes.bfloat16
N_CORES = 8
SEQ = 2048
DIM = 4096
HD = 128          # head dim
NQH = 4           # Q heads per core
QCOLS = NQH * HD  # 512
SM_SCALE = 1.0 / float(np.sqrt(HD))

_cache = {}


def _build_nc(reps: int = 1):
    import concourse.bacc as bacc
    import concourse.mybir as mybir
    import concourse.tile as tile
    import concourse.masks as masks

    dt = mybir.dt
    Alu = mybir.AluOpType
    Act = mybir.ActivationFunctionType

    nc = bacc.Bacc("TRN2", target_bir_lowering=False, debug=False)

    xT_e = nc.declare_dram_parameter("xT", [DIM, SEQ], dt.bfloat16, isOutput=False)
    wq_e = nc.declare_dram_parameter("wq", [DIM, QCOLS], dt.bfloat16, isOutput=False)
    wk_e = nc.declare_dram_parameter("wk", [DIM, HD], dt.bfloat16, isOutput=False)
    wv_e = nc.declare_dram_parameter("wv", [DIM, HD], dt.bfloat16, isOutput=False)
    wo_e = nc.declare_dram_parameter("wo", [DIM, QCOLS], dt.bfloat16, isOutput=False)
    cs_e = nc.declare_dram_parameter("cs", [256, SEQ], dt.bfloat16, isOutput=False)
    # out is transposed: out^T[oc, s]; host transposes back
    out_e = nc.declare_dram_parameter("out", [QCOLS, SEQ], dt.float32, isOutput=True)

    ag_q = [nc.dram_tensor(f"ag{i}", [DIM, 512], dt.bfloat16, addr_space="Shared")
            for i in range(4)]

    NSB = SEQ // 512   # 4 seq blocks of 512
    NCH = DIM // 128   # 32 contraction chunks
    NG = NCH // 4      # 8 four-chunk groups

    with tile.TileContext(nc) as tc:
        with (
            tc.tile_pool(name="persist", bufs=1) as pp,
            tc.tile_pool(name="dram", bufs=1, space="DRAM") as dramp,
            tc.tile_pool(name="xtp", bufs=8) as xtp,
            tc.tile_pool(name="qbfp", bufs=2) as qbfp,
            tc.tile_pool(name="vtmp", bufs=2) as vtp,
            tc.tile_pool(name="ptp", bufs=5) as ptp,
            tc.tile_pool(name="denp", bufs=2) as denp,
            tc.tile_pool(name="recp", bufs=2) as recp,
            tc.tile_pool(name="atp", bufs=3) as atp,
            tc.tile_pool(name="outp", bufs=2) as outp,
            tc.tile_pool(name="psumA", bufs=2, space="PSUM") as psA,
            tc.tile_pool(name="psumS", bufs=2, space="PSUM") as psS,
            tc.tile_pool(name="psumO", bufs=2, space="PSUM") as psO,
            tc.tile_pool(name="psumW", bufs=2, space="PSUM") as psW,
        ):
            # ---- persistent SBUF tensors ----
            wq_g = [pp.tile([128, 4 * QCOLS], dt.bfloat16, name=f"wqg{g}") for g in range(NG)]
            wk_g = [pp.tile([128, 4 * HD], dt.bfloat16, name=f"wkg{g}") for g in range(NG)]
            wv_g = [pp.tile([128, 4 * HD], dt.bfloat16, name=f"wvg{g}") for g in range(NG)]
            wo_g = [pp.tile([128, 4 * QCOLS], dt.bfloat16, name=f"wog{g}") for g in range(NG)]
            wq_sb = [wq_g[c // 4][:, (c % 4) * QCOLS:(c % 4 + 1) * QCOLS] for c in range(NCH)]
            wk_sb = [wk_g[c // 4][:, (c % 4) * HD:(c % 4 + 1) * HD] for c in range(NCH)]
            wv_sb = [wv_g[c // 4][:, (c % 4) * HD:(c % 4 + 1) * HD] for c in range(NCH)]
            wo_sb = [wo_g[c // 4][:, (c % 4) * QCOLS:(c % 4 + 1) * QCOLS] for c in range(NCH)]
            cos_sb = pp.tile([128, SEQ], dt.bfloat16)     # cos duplicated in both halves
            sin_sb = pp.tile([128, SEQ], dt.bfloat16)     # sin duplicated in both halves
            tri01 = pp.tile([128, 128], dt.bfloat16)      # 1 iff k <= q
            ident = pp.tile([128, 128], dt.bfloat16)
            ones_col = pp.tile([128, 1], dt.bfloat16)
            # per-block RoPE'd tensors (exact deps for the fused schedule)
            qrope_t = [[pp.tile([128, 512], dt.bfloat16, name=f"qr{h}_{sb}")
                        for sb in range(NSB)] for h in range(NQH)]
            krope_t = [pp.tile([128, 512], dt.bfloat16, name=f"kr{sb}") for sb in range(NSB)]
            v_t = [pp.tile([128, 512], dt.bfloat16, name=f"vt{sb}") for sb in range(NSB)]
            oTh = [[pp.tile([128, 1024], dt.bfloat16, name=f"oT{h}_{half}")
                    for half in range(2)] for h in range(NQH)]

            for g in range(NG):
                gsl = slice(g * 512, (g + 1) * 512)
                nc.scalar.dma_start(wq_g[g][:].rearrange("p (c m) -> p c m", c=4),
                                    wq_e.ap()[gsl, :].rearrange("(c p) m -> p c m", p=128))
                nc.scalar.dma_start(wk_g[g][:].rearrange("p (c m) -> p c m", c=4),
                                    wk_e.ap()[gsl, :].rearrange("(c p) m -> p c m", p=128))
                nc.scalar.dma_start(wv_g[g][:].rearrange("p (c m) -> p c m", c=4),
                                    wv_e.ap()[gsl, :].rearrange("(c p) m -> p c m", p=128))
                if g == 0:
                    nc.scalar.dma_start(cos_sb[:], cs_e.ap()[0:128, :])
                    nc.scalar.dma_start(sin_sb[:], cs_e.ap()[128:256, :])

            nc.gpsimd.memset(tri01[:], 1.0)
            nc.gpsimd.affine_select(
                out=tri01[:], in_=tri01[:], compare_op=Alu.is_ge, fill=0.0,
                base=0, pattern=[[1, 128]], channel_multiplier=-1,
            )
            masks.make_identity(nc, ident[:])
            nc.gpsimd.memset(ones_col[:], 1.0)

            agins = [dramp.tile([QCOLS, 512], dt.bfloat16, name=f"agin{i}")
                     for i in range(4)]

            for _rep in range(reps):
                # ======== per-rep emission via interleaved unit lists ========

                xts = {}  # (sb) -> list of 8 xt4 tiles

                def a_units(sb):
                    """Yield closures for block sb's QKV projections (3 passes)."""
                    sl = slice(sb * 512, (sb + 1) * 512)
                    units = []

                    def mk_pass(p):
                        # pass targets: 0 -> q0,q1 ; 1 -> q2,q3 ; 2 -> k,v
                        state = {}

                        def start():
                            state["t0"] = psA.tile([128, 512], dt.float32, name="qk")
                            state["t1"] = psA.tile([128, 512], dt.float32, name="qk")

                        def group(g):
                            def run():
                                if p == 0:
                                    if g == 0:
                                        start()
                                        xts[sb] = [None] * NG
                                    xt4 = xtp.tile([128, 4 * 512], dt.bfloat16, name="xt4")
                                    xts[sb][g] = xt4
                                    nc.sync.dma_start(
                                        xt4[:].rearrange("p (c s) -> p c s", c=4),
                                        xT_e.ap()[g * 512:(g + 1) * 512, sl]
                                        .rearrange("(c p) s -> p c s", p=128))
                                elif g == 0:
                                    start()
                                xt4 = xts[sb][g]
                                for cc in range(4):
                                    c = g * 4 + cc
                                    xt = xt4[:, cc * 512:(cc + 1) * 512]
                                    st, sp = (c == 0), (c == NCH - 1)
                                    if p == 0:
                                        w0s, w1s = wq_sb[c][:, 0:128], wq_sb[c][:, 128:256]
                                    elif p == 1:
                                        w0s, w1s = wq_sb[c][:, 256:384], wq_sb[c][:, 384:512]
                                    else:
                                        w0s, w1s = wk_sb[c], wv_sb[c]
                                    nc.tensor.matmul(state["t0"][:], w0s, xt, start=st,
                                                     stop=sp, skip_group_check=True)
                                    nc.tensor.matmul(state["t1"][:], w1s, xt, start=st,
                                                     stop=sp, skip_group_check=True)
                            return run

                        def rope(ps_key, dst):
                            def run():
                                ps = state[ps_key]
                                qbf = qbfp.tile([128, 512], dt.bfloat16, name="qbf")
                                nc.scalar.copy(qbf[:], ps[:])
                                tr_c = qbfp.tile([64, 512], dt.bfloat16, name="tr_c")
                                ti_s = qbfp.tile([64, 512], dt.bfloat16, name="ti_s")
                                tr_s = qbfp.tile([64, 512], dt.bfloat16, name="tr_s")
                                ti_c = qbfp.tile([64, 512], dt.bfloat16, name="ti_c")
                                nc.vector.tensor_mul(tr_c[:], qbf[0:64, :], cos_sb[0:64, sl])
                                nc.vector.tensor_mul(ti_s[:], qbf[64:128, :], sin_sb[64:128, sl])
                                nc.vector.tensor_sub(dst[0:64, :], tr_c[:], ti_s[:])
                                nc.vector.tensor_mul(tr_s[:], qbf[0:64, :], sin_sb[0:64, sl])
                                nc.vector.tensor_mul(ti_c[:], qbf[64:128, :], cos_sb[64:128, sl])
                                nc.vector.tensor_add(dst[64:128, :], tr_s[:], ti_c[:])
                            return run

                        def vtrans():
                            def run():
                                vT_sb = vtp.tile([128, 512], dt.bfloat16, name="vT_sb")
                                nc.scalar.copy(vT_sb[:], state["t1"][:])
                                for t in range(4):
                                    tp = psS.tile([128, 128], dt.bfloat16, name="sps")
                                    nc.tensor.transpose(tp[:], vT_sb[:, t * 128:(t + 1) * 128],
                                                        ident[:])
                                    nc.scalar.copy(v_t[sb][:, t * 128:(t + 1) * 128], tp[:])
                            return run

                        for g in range(NG):
                            units.append(group(g))
                        if p == 0:
                            units.append(rope("t0", qrope_t[0][sb]))
                            units.append(rope("t1", qrope_t[1][sb]))
                        elif p == 1:
                            units.append(rope("t0", qrope_t[2][sb]))
                            units.append(rope("t1", qrope_t[3][sb]))
                        else:
                            units.append(rope("t0", krope_t[sb]))
                            units.append(vtrans())

                    for p in range(3):
                        mk_pass(p)
                    return units

                def b_units(qb):
                    """Yield closures for attention of q-block qb (4 heads serial)."""
                    n_k = 4 * (qb + 1)
                    half = qb // 2
                    lql = slice((qb % 2) * 512, (qb % 2) * 512 + 512)
                    units = []

                    def mk_head(h):
                        state = {}

                        def step(kt):
                            def run():
                                if kt == 0:
                                    state["ops"] = psO.tile([128, 512], dt.float32, name="ops")
                                    state["dacc"] = [None, None]
                                ops = state["ops"]
                                o_idx = kt - 4 * qb
                                w0 = 128 * o_idx if o_idx > 0 else 0
                                wsl = slice(w0, 512)
                                sps = psS.tile([128, 512], dt.float32, name="sps")
                                nc.tensor.matmul(
                                    sps[:, wsl],
                                    krope_t[kt // 4][:, (kt % 4) * 128:(kt % 4 + 1) * 128],
                                    qrope_t[h][qb][:, wsl], start=True, stop=True,
                                    skip_group_check=True)
                                pt = ptp.tile([128, 512], dt.bfloat16, name="pt")
                                nc.scalar.activation(pt[:, wsl], sps[:, wsl], Act.Exp,
                                                     scale=SM_SCALE)
                                if o_idx >= 0:
                                    nc.vector.tensor_mul(pt[:, w0:w0 + 128],
                                                         pt[:, w0:w0 + 128], tri01[:])
                                nc.tensor.matmul(
                                    ops[:, wsl],
                                    v_t[kt // 4][:, (kt % 4) * 128:(kt % 4 + 1) * 128],
                                    pt[:, wsl], start=(kt == 0), stop=(kt == n_k - 1),
                                    skip_group_check=True)
                                j = kt % 2 if qb > 0 else 0
                                first = (kt == j) if qb > 0 else (kt == 0)
                                if first:
                                    d = denp.tile([128, 512], dt.bfloat16, name=f"dacc{j}")
                                    state["dacc"][j] = d
                                    nc.vector.tensor_copy(d[:], pt[:])
                                else:
                                    d = state["dacc"][j]
                                    nc.vector.tensor_add(d[:, wsl], d[:, wsl], pt[:, wsl])
                            return run

                        def finalize():
                            def run():
                                ops = state["ops"]
                                dacc = state["dacc"]
                                dsum = psS.tile([128, 512], dt.float32, name="sps")
                                nc.tensor.matmul(dsum[0:1, :], ones_col[:], dacc[0][:],
                                                 start=True, stop=(dacc[1] is None),
                                                 skip_group_check=True)
                                if dacc[1] is not None:
                                    nc.tensor.matmul(dsum[0:1, :], ones_col[:], dacc[1][:],
                                                     start=False, stop=True,
                                                     skip_group_check=True)
                                rrow = recp.tile([1, 512], dt.float32, name="rrow")
                                nc.vector.reciprocal_approx_fast(rrow[:], dsum[0:1, :])
                                rbf = recp.tile([1, 512], dt.bfloat16, name="rbf")
                                nc.vector.tensor_copy(rbf[:], rrow[:])
                                dbc = recp.tile([128, 512], dt.bfloat16, name="dbc")
                                nc.gpsimd.partition_broadcast(dbc[:], rbf[:])
                                nc.vector.tensor_mul(oTh[h][half][:, lql], ops[:], dbc[:])
                                nc.scalar.dma_start(
                                    agins[qb][h * 128:(h + 1) * 128, :],
                                    oTh[h][half][:, lql])
                            return run

                        for kt in range(n_k):
                            units.append(step(kt))
                        units.append(finalize())

                    for h in range(NQH):
                        mk_head(h)
                    return units

                def ag_unit(qb):
                    def run():
                        nc.gpsimd.collective_compute(
                            "AllGather",
                            mybir.AluOpType.bypass,
                            replica_groups=[list(range(N_CORES))],
                            ins=[agins[qb].opt()],
                            outs=[ag_q[qb][:]],
                        )
                    return run

                def d_units(qs):
                    """Flipped wo GEMM segment qs: out^T[oc, qs*512:+512], 4 accums."""
                    units = []
                    state = {}

                    def group(g):
                        def run():
                            if g == 0:
                                state["w"] = [
                                    psA.tile([128, 512], dt.float32, name="qk"),
                                    psA.tile([128, 512], dt.float32, name="qk"),
                                    psW.tile([128, 512], dt.float32, name="wops"),
                                    psW.tile([128, 512], dt.float32, name="wops"),
                                ]
                            at4 = atp.tile([128, 4 * 512], dt.bfloat16, name="at4")
                            nc.sync.dma_start(
                                at4[:].rearrange("p (c s) -> p c s", c=4),
                                ag_q[qs][g * 512:(g + 1) * 512, :]
                                .rearrange("(c p) s -> p c s", p=128))
                            for cc in range(4):
                                c = g * 4 + cc
                                at = at4[:, cc * 512:(cc + 1) * 512]
                                st, sp = (c == 0), (c == NCH - 1)
                                for oc in range(4):
                                    nc.tensor.matmul(
                                        state["w"][oc][:],
                                        wo_sb[c][:, oc * 128:(oc + 1) * 128],
                                        at, start=st, stop=sp, skip_group_check=True)
                        return run

                    def drain(oc):
                        def run():
                            outsb = outp.tile([128, 512], dt.float32, name="outsb")
                            nc.scalar.copy(outsb[:], state["w"][oc][:])
                            col0 = qs * 512
                            nc.scalar.dma_start(
                                out_e.ap()[oc * 128:(oc + 1) * 128, col0:col0 + 512],
                                outsb[:])
                        return run

                    for g in range(NG):
                        units.append(group(g))
                    for oc in range(4):
                        units.append(drain(oc))
                    return units

                def weave(primary, secondary):
                    """Merge two unit lists evenly, preserving each list's order."""
                    if not secondary:
                        return list(primary)
                    out = []
                    n, m = len(primary), len(secondary)
                    si = 0
                    for i, u in enumerate(primary):
                        out.append(u)
                        want = (i + 1) * m // n
                        while si < want:
                            out.append(secondary[si])
                            si += 1
                    out.extend(secondary[si:])
                    return out

                # phase 0: A0
                for u in a_units(0):
                    u()
                # phases 1..3: A(sb) woven with B(sb-1); gather qb right after
                for sb in range(1, NSB):
                    for u in weave(a_units(sb), b_units(sb - 1)):
                        u()
                    ag_unit(sb - 1)()
                    if _rep == 0 and sb == 3:
                        for g in range(NG):
                            nc.scalar.dma_start(
                                wo_g[g][:].rearrange("p (c m) -> p c m", c=4),
                                wo_e.ap()[g * 512:(g + 1) * 512, :]
                                .rearrange("(c p) m -> p c m", p=128))
                # phase 4: B3 (front-loaded, dense) woven with D segment 0
                for u in weave(b_units(3), d_units(0)):
                    u()
                ag_unit(3)()
                # phase 5: D segments 1-3
                for qs in (1, 2, 3):
                    for u in d_units(qs):
                        u()

    nc.compile()
    return nc


def _prep_inputs(x, wq, wk, wv, wo):
    """Host-side sharding/layout prep. Returns per-core in_maps."""
    x2 = np.asarray(x, dtype=np.float32).reshape(SEQ, DIM)
    xT = np.ascontiguousarray(x2.T).astype(BF16)

    # permutation: within each head, even dims then odd dims (RoPE pair layout)
    perm_head = np.concatenate([np.arange(0, HD, 2), np.arange(1, HD, 2)])
    qperm = np.concatenate([g * HD + perm_head for g in range(32)])   # 32 Q heads
    kperm = np.concatenate([g * HD + perm_head for g in range(8)])    # 8 KV heads
    wq_p = np.asarray(wq, dtype=np.float32)[:, qperm].astype(BF16)
    wk_p = np.asarray(wk, dtype=np.float32)[:, kperm].astype(BF16)
    wv_b = np.asarray(wv, dtype=np.float32).astype(BF16)
    wo_b = np.asarray(wo, dtype=np.float32).astype(BF16)

    # RoPE tables: cos/sin[j, s], j = pair index 0..63
    inv_freq = 1.0 / (10000.0 ** (np.arange(0, HD, 2, dtype=np.float64) / HD))
    ang = inv_freq[:, None] * np.arange(SEQ, dtype=np.float64)[None, :]
    cosd = np.cos(ang)
    sind = np.sin(ang)
    cs = np.concatenate([cosd, cosd, sind, sind]).astype(BF16)

    in_maps = []
    for i in range(N_CORES):
        in_maps.append({
            "xT": xT,
            "wq": np.ascontiguousarray(wq_p[:, i * QCOLS:(i + 1) * QCOLS]),
            "wk": np.ascontiguousarray(wk_p[:, i * HD:(i + 1) * HD]),
            "wv": np.ascontiguousarray(wv_b[:, i * HD:(i + 1) * HD]),
            "wo": np.ascontiguousarray(wo_b[:, i * QCOLS:(i + 1) * QCOLS]),
            "cs": cs,
        })
    return in_maps


def _get_nc(reps: int = 1):
    key = ("nc", reps)
    if key not in _cache:
        _cache[key] = _build_nc(reps)
    return _cache[key]


def unshard(per_core_out):
    """per_core_out: list of 8 arrays [QCOLS, SEQ] (out^T) -> [1, SEQ, DIM]."""
    outT = np.concatenate(per_core_out, axis=0)  # [DIM, SEQ]
    return np.ascontiguousarray(outT.T).reshape(1, SEQ, DIM).astype(np.float32)


def kernel(x, wq, wk, wv, wo, start_pos=0, **_ignored):
    from concourse.bass_utils import run_bass_kernel_spmd

    nc = _get_nc()
    in_maps = _prep_inputs(x, wq, wk, wv, wo)
    res = run_bass_kernel_spmd(nc, in_maps, core_ids=list(range(N_CORES)))
    return unshard([res.results[i]["out"] for i in range(N_CORES)])


# revision 13
# speedup vs baseline: 1.1906x; 1.0119x over previous
"""Llama-3-8B-style GQA attention layer (bsz=1, seq=2048, dim=4096) on 8 TRN2 NeuronCores.

Tensor-parallel over heads: core i owns Q heads 4i..4i+3 and KV head i.

Fused pipeline (single static pool set, no per-rep pool barriers):
  Block sb's QKV projections (3 psum-light passes: {q0,q1},{q2,q3},{k,v}) are
  interleaved in program order with attention of block sb-1, so the PE fills
  softmax-latency stalls with projection matmuls and stays HAM-warm.
  Attention q-block 3 is interleaved with the (flipped) wo GEMM half 0.
  Stage C: AllGather of normalized O^T (bf16) in two s-halves; AG1 overlaps
  blocks 2-3, AG2 overlaps wo-half-0.
  Stage D: wo GEMM flipped (wo chunk stationary, O^T moving) producing
  out^T[oc, s] with 4 psum accumulators; host transposes.
  Softmax denominator: rank-1 PE matmuls -> reciprocal_approx_fast on the
  [1,512] row -> bf16 -> gpsimd partition_broadcast -> one DVE multiply.
"""
import numpy as np
import ml_dtypes

BF16 = ml_dtypes.bfloat16
N_CORES = 8
SEQ = 2048
DIM = 4096
HD = 128          # head dim
NQH = 4           # Q heads per core
QCOLS = NQH * HD  # 512
SM_SCALE = 1.0 / float(np.sqrt(HD))

_cache = {}


def _build_nc(reps: int = 1):
    import concourse.bacc as bacc
    import concourse.mybir as mybir
    import concourse.tile as tile
    import concourse.masks as masks

    dt = mybir.dt
    Alu = mybir.AluOpType
    Act = mybir.ActivationFunctionType

    nc = bacc.Bacc("TRN2", target_bir_lowering=False, debug=False)

    xT_e = nc.declare_dram_parameter("xT", [DIM, SEQ], dt.bfloat16, isOutput=False)
    wq_e = nc.declare_dram_parameter("wq", [DIM, QCOLS], dt.bfloat16, isOutput=False)
    wk_e = nc.declare_dram_parameter("wk", [DIM, HD], dt.bfloat16, isOutput=False)
    wv_e = nc.declare_dram_parameter("wv", [DIM, HD], dt.bfloat16, isOutput=False)
    wo_e = nc.declare_dram_parameter("wo", [DIM, QCOLS], dt.bfloat16, isOutput=False)
    cs_e = nc.declare_dram_parameter("cs", [256, SEQ], dt.bfloat16, isOutput=False)
    # out is transposed: out^T[oc, s]; host transposes back
    out_e = nc.declare_dram_parameter("out", [QCOLS, SEQ], dt.float32, isOutput=True)

    ag_q = [nc.dram_tensor(f"ag{i}", [DIM, 512], dt.bfloat16, addr_space="Shared")
            for i in range(4)]

    NSB = SEQ // 512   # 4 seq blocks of 512
    NCH = DIM // 128   # 32 contraction chunks
    NG = NCH // 4      # 8 four-chunk groups

    with tile.TileContext(nc) as tc:
        with (
            tc.tile_pool(name="persist", bufs=1) as pp,
            tc.tile_pool(name="dram", bufs=1, space="DRAM") as dramp,
            tc.tile_pool(name="xtp", bufs=8) as xtp,
            tc.tile_pool(name="qbfp", bufs=2) as qbfp,
            tc.tile_pool(name="vtmp", bufs=2) as vtp,
            tc.tile_pool(name="ptp", bufs=5) as ptp,
            tc.tile_pool(name="denp", bufs=2) as denp,
            tc.tile_pool(name="recp", bufs=2) as recp,
            tc.tile_pool(name="atp", bufs=3) as atp,
            tc.tile_pool(name="outp", bufs=2) as outp,
            tc.tile_pool(name="psumA", bufs=2, space="PSUM") as psA,
            tc.tile_pool(name="psumS", bufs=2, space="PSUM") as psS,
            tc.tile_pool(name="psumO", bufs=2, space="PSUM") as psO,
            tc.tile_pool(name="psumW", bufs=2, space="PSUM") as psW,
        ):
            # ---- persistent SBUF tensors ----
            wq_g = [pp.tile([128, 4 * QCOLS], dt.bfloat16, name=f"wqg{g}") for g in range(NG)]
            wk_g = [pp.tile([128, 4 * HD], dt.bfloat16, name=f"wkg{g}") for g in range(NG)]
            wv_g = [pp.tile([128, 4 * HD], dt.bfloat16, name=f"wvg{g}") for g in range(NG)]
            wo_g = [pp.tile([128, 4 * QCOLS], dt.bfloat16, name=f"wog{g}") for g in range(NG)]
            wq_sb = [wq_g[c // 4][:, (c % 4) * QCOLS:(c % 4 + 1) * QCOLS] for c in range(NCH)]
            wk_sb = [wk_g[c // 4][:, (c % 4) * HD:(c % 4 + 1) * HD] for c in range(NCH)]
            wv_sb = [wv_g[c // 4][:, (c % 4) * HD:(c % 4 + 1) * HD] for c in range(NCH)]
            wo_sb = [wo_g[c // 4][:, (c % 4) * QCOLS:(c % 4 + 1) * QCOLS] for c in range(NCH)]
            cos_sb = pp.tile([128, SEQ], dt.bfloat16)     # cos duplicated in both halves
            sin_sb = pp.tile([128, SEQ], dt.bfloat16)     # sin duplicated in both halves
            tri01 = pp.tile([128, 128], dt.bfloat16)      # 1 iff k <= q
            ident = pp.tile([128, 128], dt.bfloat16)
            # per-block RoPE'd tensors (exact deps for the fused schedule)
            qrope_t = [[pp.tile([128, 512], dt.bfloat16, name=f"qr{h}_{sb}")
                        for sb in range(NSB)] for h in range(NQH)]
            krope_t = [pp.tile([128, 512], dt.bfloat16, name=f"kr{sb}") for sb in range(NSB)]
            v_t = [pp.tile([128, 512], dt.bfloat16, name=f"vt{sb}") for sb in range(NSB)]
            oTh = [[pp.tile([128, 1024], dt.bfloat16, name=f"oT{h}_{half}")
                    for half in range(2)] for h in range(NQH)]

            for g in range(NG):
                gsl = slice(g * 512, (g + 1) * 512)
                nc.scalar.dma_start(wq_g[g][:].rearrange("p (c m) -> p c m", c=4),
                                    wq_e.ap()[gsl, :].rearrange("(c p) m -> p c m", p=128))
                nc.scalar.dma_start(wk_g[g][:].rearrange("p (c m) -> p c m", c=4),
                                    wk_e.ap()[gsl, :].rearrange("(c p) m -> p c m", p=128))
                nc.scalar.dma_start(wv_g[g][:].rearrange("p (c m) -> p c m", c=4),
                                    wv_e.ap()[gsl, :].rearrange("(c p) m -> p c m", p=128))
                if g == 0:
                    nc.scalar.dma_start(cos_sb[:], cs_e.ap()[0:128, :])
                    nc.scalar.dma_start(sin_sb[:], cs_e.ap()[128:256, :])

            nc.gpsimd.memset(tri01[:], 1.0)
            nc.gpsimd.affine_select(
                out=tri01[:], in_=tri01[:], compare_op=Alu.is_ge, fill=0.0,
                base=0, pattern=[[1, 128]], channel_multiplier=-1,
            )
            masks.make_identity(nc, ident[:])

            agins = [dramp.tile([QCOLS, 512], dt.bfloat16, name=f"agin{i}")
                     for i in range(4)]

            for _rep in range(reps):
                # ======== per-rep emission via interleaved unit lists ========

                xts = {}  # (sb) -> list of 8 xt4 tiles

                def a_units(sb):
                    """Yield closures for block sb's QKV projections (3 passes)."""
                    sl = slice(sb * 512, (sb + 1) * 512)
                    units = []

                    def mk_pass(p):
                        # pass targets: 0 -> q0,q1 ; 1 -> q2,q3 ; 2 -> k,v
                        state = {}

                        def start():
                            state["t0"] = psA.tile([128, 512], dt.float32, name="qk")
                            state["t1"] = psA.tile([128, 512], dt.float32, name="qk")

                        def group(g):
                            def run():
                                if p == 0:
                                    if g == 0:
                                        start()
                                        xts[sb] = [None] * NG
                                    xt4 = xtp.tile([128, 4 * 512], dt.bfloat16, name="xt4")
                                    xts[sb][g] = xt4
                                    nc.sync.dma_start(
                                        xt4[:].rearrange("p (c s) -> p c s", c=4),
                                        xT_e.ap()[g * 512:(g + 1) * 512, sl]
                                        .rearrange("(c p) s -> p c s", p=128))
                                elif g == 0:
                                    start()
                                xt4 = xts[sb][g]
                                for cc in range(4):
                                    c = g * 4 + cc
                                    xt = xt4[:, cc * 512:(cc + 1) * 512]
                                    st, sp = (c == 0), (c == NCH - 1)
                                    if p == 0:
                                        w0s, w1s = wq_sb[c][:, 0:128], wq_sb[c][:, 128:256]
                                    elif p == 1:
                                        w0s, w1s = wq_sb[c][:, 256:384], wq_sb[c][:, 384:512]
                                    else:
                                        w0s, w1s = wk_sb[c], wv_sb[c]
                                    nc.tensor.matmul(state["t0"][:], w0s, xt, start=st,
                                                     stop=sp, skip_group_check=True)
                                    nc.tensor.matmul(state["t1"][:], w1s, xt, start=st,
                                                     stop=sp, skip_group_check=True)
                            return run

                        def rope(ps_key, dst):
                            def run():
                                ps = state[ps_key]
                                qbf = qbfp.tile([128, 512], dt.bfloat16, name="qbf")
                                nc.scalar.copy(qbf[:], ps[:])
                                tr_c = qbfp.tile([64, 512], dt.bfloat16, name="tr_c")
                                ti_s = qbfp.tile([64, 512], dt.bfloat16, name="ti_s")
                                tr_s = qbfp.tile([64, 512], dt.bfloat16, name="tr_s")
                                ti_c = qbfp.tile([64, 512], dt.bfloat16, name="ti_c")
                                nc.vector.tensor_mul(tr_c[:], qbf[0:64, :], cos_sb[0:64, sl])
                                nc.vector.tensor_mul(ti_s[:], qbf[64:128, :], sin_sb[64:128, sl])
                                nc.vector.tensor_sub(dst[0:64, :], tr_c[:], ti_s[:])
                                nc.vector.tensor_mul(tr_s[:], qbf[0:64, :], sin_sb[0:64, sl])
                                nc.vector.tensor_mul(ti_c[:], qbf[64:128, :], cos_sb[64:128, sl])
                                nc.vector.tensor_add(dst[64:128, :], tr_s[:], ti_c[:])
                            return run

                        def vtrans():
                            def run():
                                vT_sb = vtp.tile([128, 512], dt.bfloat16, name="vT_sb")
                                nc.scalar.copy(vT_sb[:], state["t1"][:])
                                for t in range(4):
                                    tp = psS.tile([128, 128], dt.bfloat16, name="sps")
                                    nc.tensor.transpose(tp[:], vT_sb[:, t * 128:(t + 1) * 128],
                                                        ident[:])
                                    nc.scalar.copy(v_t[sb][:, t * 128:(t + 1) * 128], tp[:])
                            return run

                        for g in range(NG):
                            units.append(group(g))
                        if p == 0:
                            units.append(rope("t0", qrope_t[0][sb]))
                            units.append(rope("t1", qrope_t[1][sb]))
                        elif p == 1:
                            units.append(rope("t0", qrope_t[2][sb]))
                            units.append(rope("t1", qrope_t[3][sb]))
                        else:
                            units.append(rope("t0", krope_t[sb]))
                            units.append(vtrans())

                    for p in range(3):
                        mk_pass(p)
                    return units

                def b_units(qb):
                    """Yield closures for attention of q-block qb (4 heads serial)."""
                    n_k = 4 * (qb + 1)
                    half = qb // 2
                    lql = slice((qb % 2) * 512, (qb % 2) * 512 + 512)
                    units = []

                    def mk_head(h):
                        state = {}

                        def step(kt):
                            def run():
                                if kt == 0:
                                    state["ops"] = psO.tile([128, 512], dt.float32, name="ops")
                                    state["dacc"] = [None, None]
                                ops = state["ops"]
                                o_idx = kt - 4 * qb
                                w0 = 128 * o_idx if o_idx > 0 else 0
                                wsl = slice(w0, 512)
                                sps = psS.tile([128, 512], dt.float32, name="sps")
                                nc.tensor.matmul(
                                    sps[:, wsl],
                                    krope_t[kt // 4][:, (kt % 4) * 128:(kt % 4 + 1) * 128],
                                    qrope_t[h][qb][:, wsl], start=True, stop=True,
                                    skip_group_check=True)
                                pt = ptp.tile([128, 512], dt.bfloat16, name="pt")
                                nc.scalar.activation(pt[:, wsl], sps[:, wsl], Act.Exp,
                                                     scale=SM_SCALE)
                                if o_idx >= 0:
                                    nc.vector.tensor_mul(pt[:, w0:w0 + 128],
                                                         pt[:, w0:w0 + 128], tri01[:])
                                nc.tensor.matmul(
                                    ops[:, wsl],
                                    v_t[kt // 4][:, (kt % 4) * 128:(kt % 4 + 1) * 128],
                                    pt[:, wsl], start=(kt == 0), stop=(kt == n_k - 1),
                                    skip_group_check=True)
                                j = kt % 2 if qb > 0 else 0
                                first = (kt == j) if qb > 0 else (kt == 0)
                                if first:
                                    d = denp.tile([128, 512], dt.bfloat16, name=f"dacc{j}")
                                    state["dacc"][j] = d
                                    nc.vector.tensor_copy(d[:], pt[:])
                                else:
                                    d = state["dacc"][j]
                                    nc.vector.tensor_add(d[:, wsl], d[:, wsl], pt[:, wsl])
                            return run

                        def finalize():
                            def run():
                                from concourse import bass_isa
                                ops = state["ops"]
                                dacc = state["dacc"]
                                if dacc[1] is not None:
                                    nc.vector.tensor_add(dacc[0][:], dacc[0][:], dacc[1][:])
                                red = recp.tile([128, 512], dt.float32, name="red")
                                nc.gpsimd.partition_all_reduce(
                                    red[:], dacc[0][:], channels=128,
                                    reduce_op=bass_isa.ReduceOp.add)
                                rec = recp.tile([128, 512], dt.float32, name="rec")
                                nc.vector.reciprocal_approx_fast(rec[:], red[:])
                                nc.vector.tensor_mul(oTh[h][half][:, lql], ops[:], rec[:])
                                nc.scalar.dma_start(
                                    agins[qb][h * 128:(h + 1) * 128, :],
                                    oTh[h][half][:, lql])
                            return run

                        for kt in range(n_k):
                            units.append(step(kt))
                        units.append(finalize())

                    for h in range(NQH):
                        mk_head(h)
                    return units

                def ag_unit(qb):
                    def run():
                        nc.gpsimd.collective_compute(
                            "AllGather",
                            mybir.AluOpType.bypass,
                            replica_groups=[list(range(N_CORES))],
                            ins=[agins[qb].opt()],
                            outs=[ag_q[qb][:]],
                        )
                    return run

                def d_units(qs):
                    """Flipped wo GEMM segment qs: out^T[oc, qs*512:+512], 4 accums."""
                    units = []
                    state = {}

                    def group(g):
                        def run():
                            if g == 0:
                                state["w"] = [
                                    psA.tile([128, 512], dt.float32, name="qk"),
                                    psA.tile([128, 512], dt.float32, name="qk"),
                                    psW.tile([128, 512], dt.float32, name="wops"),
                                    psW.tile([128, 512], dt.float32, name="wops"),
                                ]
                            at4 = atp.tile([128, 4 * 512], dt.bfloat16, name="at4")
                            nc.sync.dma_start(
                                at4[:].rearrange("p (c s) -> p c s", c=4),
                                ag_q[qs][g * 512:(g + 1) * 512, :]
                                .rearrange("(c p) s -> p c s", p=128))
                            for cc in range(4):
                                c = g * 4 + cc
                                at = at4[:, cc * 512:(cc + 1) * 512]
                                st, sp = (c == 0), (c == NCH - 1)
                                for oc in range(4):
                                    nc.tensor.matmul(
                                        state["w"][oc][:],
                                        wo_sb[c][:, oc * 128:(oc + 1) * 128],
                                        at, start=st, stop=sp, skip_group_check=True)
                        return run

                    def drain(oc):
                        def run():
                            outsb = outp.tile([128, 512], dt.float32, name="outsb")
                            nc.scalar.copy(outsb[:], state["w"][oc][:])
                            col0 = qs * 512
                            nc.scalar.dma_start(
                                out_e.ap()[oc * 128:(oc + 1) * 128, col0:col0 + 512],
                                outsb[:])
                        return run

                    for g in range(NG):
                        units.append(group(g))
                    for oc in range(4):
                        units.append(drain(oc))
                    return units

                def weave(primary, secondary):
                    """Merge two unit lists evenly, preserving each list's order."""
                    if not secondary:
                        return list(primary)
                    out = []
                    n, m = len(primary), len(secondary)
                    si = 0
                    for i, u in enumerate(primary):
                        out.append(u)
                        want = (i + 1) * m // n
                        while si < want:
                            out.append(secondary[si])
                            si += 1
                    out.extend(secondary[si:])
                    return out

                # phase 0: A0
                for u in a_units(0):
                    u()
                # phases 1..3: A(sb) woven with B(sb-1); gather qb right after
                for sb in range(1, NSB):
                    for u in weave(a_units(sb), b_units(sb - 1)):
                        u()
                    ag_unit(sb - 1)()
                    if _rep == 0 and sb == 3:
                        for g in range(NG):
                            nc.scalar.dma_start(
                                wo_g[g][:].rearrange("p (c m) -> p c m", c=4),
                                wo_e.ap()[g * 512:(g + 1) * 512, :]
                                .rearrange("(c p) m -> p c m", p=128))
                # phase 4: B3 (front-loaded, dense) woven with D segment 0
                for u in weave(b_units(3), d_units(0)):
                    u()
                ag_unit(3)()
                # phase 5: D segments 1-3
                for qs in (1, 2, 3):
                    for u in d_units(qs):
                        u()

    nc.compile()
    return nc


def _prep_inputs(x, wq, wk, wv, wo):
    """Host-side sharding/layout prep. Returns per-core in_maps."""
    x2 = np.asarray(x, dtype=np.float32).reshape(SEQ, DIM)
    xT = np.ascontiguousarray(x2.T).astype(BF16)

    # permutation: within each head, even dims then odd dims (RoPE pair layout)
    perm_head = np.concatenate([np.arange(0, HD, 2), np.arange(1, HD, 2)])
    qperm = np.concatenate([g * HD + perm_head for g in range(32)])   # 32 Q heads
    kperm = np.concatenate([g * HD + perm_head for g in range(8)])    # 8 KV heads
    wq_p = np.asarray(wq, dtype=np.float32)[:, qperm].astype(BF16)
    wk_p = np.asarray(wk, dtype=np.float32)[:, kperm].astype(BF16)
    wv_b = np.asarray(wv, dtype=np.float32).astype(BF16)
    wo_b = np.asarray(wo, dtype=np.float32).astype(BF16)

    # RoPE tables: cos/sin[j, s], j = pair index 0..63
    inv_freq = 1.0 / (10000.0 ** (np.arange(0, HD, 2, dtype=np.float64) / HD))
    ang = inv_freq[:, None] * np.arange(SEQ, dtype=np.float64)[None, :]
    cosd = np.cos(ang)
    sind = np.sin(ang)
    cs = np.concatenate([cosd, cosd, sind, sind]).astype(BF16)

    in_maps = []
    for i in range(N_CORES):
        in_maps.append({
            "xT": xT,
            "wq": np.ascontiguousarray(wq_p[:, i * QCOLS:(i + 1) * QCOLS]),
            "wk": np.ascontiguousarray(wk_p[:, i * HD:(i + 1) * HD]),
            "wv": np.ascontiguousarray(wv_b[:, i * HD:(i + 1) * HD]),
            "wo": np.ascontiguousarray(wo_b[:, i * QCOLS:(i + 1) * QCOLS]),
            "cs": cs,
        })
    return in_maps


def _get_nc(reps: int = 1):
    key = ("nc", reps)
    if key not in _cache:
        _cache[key] = _build_nc(reps)
    return _cache[key]


def unshard(per_core_out):
    """per_core_out: list of 8 arrays [QCOLS, SEQ] (out^T) -> [1, SEQ, DIM]."""
    outT = np.concatenate(per_core_out, axis=0)  # [DIM, SEQ]
    return np.ascontiguousarray(outT.T).reshape(1, SEQ, DIM).astype(np.float32)


def kernel(x, wq, wk, wv, wo, start_pos=0, **_ignored):
    from concourse.bass_utils import run_bass_kernel_spmd

    nc = _get_nc()
    in_maps = _prep_inputs(x, wq, wk, wv, wo)
    res = run_bass_kernel_spmd(nc, in_maps, core_ids=list(range(N_CORES)))
    return unshard([res.results[i]["out"] for i in range(N_CORES)])
